# revision 23
# baseline (speedup 1.0000x reference)
"""Single-head attention Trainium2 kernel (batch=8 data-parallel over 8 cores).

Reference computation (per batch element b):
    Q = x @ Wq; K = x @ Wk; V = x @ Wv          (x: [S, D], W*: [D, O])
    out = softmax(Q @ K.T * SCALE) @ V          (SCALE = 1/8, hardcoded sqrt(64))

Kernel strategy (per core, one batch element), all matmuls f32r:
  Host precomputes M = Wq @ Wk^T (exact f32), so scores = (x M) x^T and the
  device never materializes Q or K separately:
  Phase A: x -> xT via PE transposes (d on partitions).
  Phase B: G'^T = (xM)^T streamed to DRAM scratch (plays the role of Q^T);
           "K^T" is just xT, copied to phase-C-lived tiles. This removes the
           entire K projection (one of the three S*D*O matmuls) versus the
           plain QKV formulation.
  Phase C (per q-block): scoresT[ks,q] = xT-chunks @ G'T-block,
           expT = exp(SCALE*scoresT) (ACT, fused scale),
           row-sums via ones-matmul -> reciprocal,
           A^T[d,q] = x-chunks.T @ expT (reassociation: attn @ x),
           out[q,o] = (A^T).T @ Wv, normalized by reciprocal on eviction.
"""
import sys
sys.path.insert(0, "/opt/trn_rl_repo")
from contextlib import ExitStack
import numpy as np
import concourse.bass as bass
import concourse.mybir as mybir
from concourse import bacc
from concourse.tile import TileContext
from concourse.masks import make_identity

F32 = mybir.dt.float32
F32R = mybir.dt.float32r
BF16 = mybir.dt.bfloat16
EXP = mybir.ActivationFunctionType.Exp
SCALE = 1.0 / 8.0


def build_attn(S=2048, D=1024, O=1024, QB=512, compute_dtype=F32R, reps=1, phases='abcuvo'):
    CD = compute_dtype
    SB = 512
    NSB = S // SB
    DC = D // 128
    OC = O // 128
    KC = S // 128
    NQB = S // QB
    QC = QB // 128
    OH = (O + 511) // 512
    OHW = min(O, 512)

    nc = bacc.Bacc("TRN2", target_bir_lowering=False, debug=False)
    x_in = nc.dram_tensor("x", [S, D], F32, kind="ExternalInput")
    m_in = nc.dram_tensor("m", [D, O], F32, kind="ExternalInput")
    wv_in = nc.dram_tensor("wv", [D, O], F32, kind="ExternalInput")
    out_d = nc.dram_tensor("out", [S, O], F32, kind="ExternalOutput")

    def cast(ap):
        return ap.bitcast(CD) if CD != F32 else ap

    with TileContext(nc) as tc:
      for _rep in range(reps):
        top = ExitStack()
        kt_pool = top.enter_context(tc.tile_pool(name="ktp", bufs=OC * NSB))
        gq_pool = top.enter_context(tc.tile_pool(name="gqp", bufs=OC * NSB))
        const_pool = top.enter_context(tc.tile_pool(name="constp", bufs=1))

        ident_f = const_pool.tile([128, 128], F32, tag="identf")
        make_identity(nc, ident_f)
        if CD != F32:
            ident = const_pool.tile([128, 128], CD, tag="identr")
            nc.vector.tensor_copy(out=ident, in_=ident_f)
        else:
            ident = ident_f
        ones_f = const_pool.tile([128, 1], F32, tag="onesf")
        nc.gpsimd.memset(ones_f, 1.0)
        ones_b = const_pool.tile([128, 1], BF16, tag="onesb")
        nc.gpsimd.memset(ones_b, 1.0)

        kt = [[None] * NSB for _ in range(OC)]
        gqt = [[None] * NSB for _ in range(OC)]

        with ExitStack() as ph_ab:
            xn_pool = ph_ab.enter_context(tc.tile_pool(name="xnp", bufs=6))
            xt_pool = ph_ab.enter_context(tc.tile_pool(name="xtp", bufs=DC * NSB))
            w_pool = ph_ab.enter_context(tc.tile_pool(name="wp", bufs=4 * DC))
            psA = ph_ab.enter_context(tc.tile_pool(name="psA", bufs=4, space="PSUM"))
            psB = ph_ab.enter_context(tc.tile_pool(name="psB", bufs=4, space="PSUM"))

            xt = [[None] * NSB for _ in range(DC)]
            xn_sb = [None] * (SB // 128)
            _mid = (NSB + 1) // 2
            _halves = [h for h in (list(range(0, _mid)), list(range(_mid, NSB))) if h]
            for half in range(len(_halves) if 'b' in phases else 0):
                sbs = _halves[half]
                for sb in sbs:
                    for ss in range(SB // 128):
                        kc = sb * (SB // 128) + ss
                        xn_t = xn_pool.tile([128, D], CD, tag="xn", bufs=6)
                        dma_eng = nc.sync if kc % 2 == 0 else nc.gpsimd
                        dma_eng.dma_start(
                            out=xn_t, in_=cast(x_in[kc * 128 : (kc + 1) * 128, :])
                        )
                        xn_sb[ss] = xn_t
                    for dc in range(DC):
                        ps = psA.tile([128, SB], CD, tag="pst", bufs=4)
                        for s2 in range(SB // 128):
                            nc.tensor.transpose(
                                ps[:, s2 * 128 : (s2 + 1) * 128],
                                xn_sb[s2][:, dc * 128 : (dc + 1) * 128],
                                ident,
                            )
                        xt[dc][sb] = xt_pool.tile(
                            [128, SB], CD, tag="xt", bufs=DC * NSB, name=f"xt_{dc}_{sb}"
                        )
                        nc.vector.tensor_copy(out=xt[dc][sb], in_=ps)
                for oc in range(OC):
                    wq_t = w_pool.tile(
                        [128, DC * 128], CD, tag="wq", bufs=3, name=f"wq_{half}_{oc}"
                    )
                    nc.sync.dma_start(
                        out=wq_t.rearrange("p (c o) -> p c o", c=DC),
                        in_=cast(m_in[:, oc * 128 : (oc + 1) * 128]).rearrange(
                            "(c p) o -> p c o", p=128
                        ),
                    )
                    wq_col = [wq_t[:, dc * 128 : (dc + 1) * 128] for dc in range(DC)]
                    ps_qs = [psB.tile([128, SB], F32, tag="psb", bufs=4,
                                      name=f"psb_{half}_{oc}_{j}")
                             for j in range(len(sbs))]
                    for dc in range(DC):
                        for j, sb in enumerate(sbs):
                            nc.tensor.matmul(
                                ps_qs[j], wq_col[dc], xt[dc][sb],
                                start=(dc == 0), stop=(dc == DC - 1),
                            )
                    for j, sb in enumerate(sbs):
                        # G'T stays resident in SBUF as bf16 (no DRAM trip)
                        gqt[oc][sb] = gq_pool.tile(
                            [128, SB], BF16, tag="gq", bufs=OC * NSB,
                            name=f"gq_{oc}_{sb}"
                        )
                        nc.vector.tensor_copy(out=gqt[oc][sb], in_=ps_qs[j])
                        # "K^T" for scores is just x^T, rounded to bf16
                        if kt[oc][sb] is None:
                            kt_tile = kt_pool.tile(
                                [128, SB], BF16, tag="kt", bufs=OC * NSB,
                                name=f"kt_{oc}_{sb}"
                            )
                            nc.scalar.copy(out=kt_tile,
                                           in_=xt[oc][sb].bitcast(F32))
                            kt[oc][sb] = kt_tile

        with ExitStack() as ph_c:
          if 'c' in phases:
                xn2_pool = ph_c.enter_context(tc.tile_pool(name="xn2p", bufs=KC))
                wv_pool = ph_c.enter_context(tc.tile_pool(name="wvp", bufs=DC))
                exp_pool = ph_c.enter_context(tc.tile_pool(name="expp", bufs=2 * KC + 2))
                at_pool = ph_c.enter_context(tc.tile_pool(name="atp", bufs=2 * DC))
                outs_pool = ph_c.enter_context(tc.tile_pool(name="outsp", bufs=2))
                small_pool = ph_c.enter_context(tc.tile_pool(name="smallp", bufs=4 * QC))
                pcs = ph_c.enter_context(tc.tile_pool(name="pcs", bufs=3, space="PSUM"))
                pcsum = ph_c.enter_context(tc.tile_pool(name="pcsum", bufs=1, space="PSUM"))
                pca = ph_c.enter_context(tc.tile_pool(name="pca", bufs=2, space="PSUM"))
                pco = ph_c.enter_context(tc.tile_pool(name="pco", bufs=2, space="PSUM"))

                xn2 = []
                for kc in range(KC):
                    t = xn2_pool.tile([128, D], BF16, tag="xn2", bufs=KC, name=f"xn2_{kc}")
                    nc.gpsimd.dma_start(out=t, in_=x_in[kc * 128 : (kc + 1) * 128, :])
                    xn2.append(t)
                wv = []
                for dc in range(DC):
                    t = wv_pool.tile([128, O], CD, tag="wv", bufs=DC, name=f"wv_{dc}")
                    nc.gpsimd.dma_start(out=t, in_=cast(wv_in[dc * 128 : (dc + 1) * 128, :]))
                    wv.append(t)

                for qbp in range(NQB // 2):
                    qbs = [2 * qbp, 2 * qbp + 1]
                    qts = [[gqt[oc][qb] for oc in range(OC)] for qb in qbs]
                    # scoresT for both q-blocks of the pair: each kt-slice
                    # stationary is loaded once and streamed against both.
                    expT = [[], []]
                    for kc in range(KC):
                        sb, ss = kc // (SB // 128), kc % (SB // 128)
                        ps_pair = [pcs.tile([128, QB], F32, tag="pcs", bufs=3,
                                            name=f"pcs_{qbp}_{kc}_{i}")
                                   for i in range(2)]
                        for oc in range(OC):
                            st = kt[oc][sb][:, ss * 128 : (ss + 1) * 128]
                            for i in range(2):
                                nc.tensor.matmul(
                                    ps_pair[i], st, qts[i][oc],
                                    start=(oc == 0), stop=(oc == OC - 1),
                                )
                        for i in range(2):
                            e = exp_pool.tile([128, QB], BF16, tag="expT",
                                              bufs=2 * KC + 2,
                                              name=f"expT_{qbp}_{kc}_{i}")
                            nc.scalar.activation(out=e, in_=ps_pair[i],
                                                 func=EXP, scale=SCALE)
                            expT[i].append(e)
                    # A^T for both q-blocks: each x-slice stationary loaded
                    # once, streamed against both q-blocks' expT.
                    aT = [[], []]
                    for dc in range(DC):
                        pa_pair = [pca.tile([128, QB], F32, tag="pca", bufs=2,
                                            name=f"pca_{qbp}_{dc}_{i}")
                                   for i in range(2)]
                        for kc in range(KC):
                            st = xn2[kc][:, dc * 128 : (dc + 1) * 128]
                            for i in range(2):
                                nc.tensor.matmul(
                                    pa_pair[i], st, expT[i][kc],
                                    start=(kc == 0), stop=(kc == KC - 1),
                                )
                        for i in range(2):
                            a_t = at_pool.tile([128, QB], CD, tag="aT",
                                               bufs=2 * DC,
                                               name=f"aT_{qbp}_{dc}_{i}")
                            nc.vector.tensor_copy(out=a_t, in_=pa_pair[i])
                            aT[i].append(a_t)
                    # rowsums + normalized out per q-block
                    for i, qb in enumerate(qbs):
                        q0 = qb * QB
                        recips = []
                        for qc in range(QC if 'u' in phases else 0):
                            ps_sum = pcsum.tile([128, 1], F32, tag="pcsum", bufs=1)
                            for kc in range(KC):
                                nc.tensor.matmul(
                                    ps_sum,
                                    expT[i][kc][:, qc * 128 : (qc + 1) * 128],
                                    ones_b,
                                    start=(kc == 0), stop=(kc == KC - 1),
                                )
                            rc = small_pool.tile([128, 1], F32, tag="recip",
                                                 bufs=4 * QC)
                            nc.vector.reciprocal(out=rc, in_=ps_sum)
                            recips.append(rc)
                        for qc in range(QC if 'o' in phases else 0):
                            ps_os = [pco.tile([128, OHW], F32, tag="pco", bufs=2,
                                              name=f"pco_{qb}_{qc}_{oh}")
                                     for oh in range(OH)]
                            for dc in range(DC):
                                st = aT[i][dc][:, qc * 128 : (qc + 1) * 128]
                                for oh in range(OH):
                                    nc.tensor.matmul(
                                        ps_os[oh], st,
                                        wv[dc][:, oh * OHW : (oh + 1) * OHW],
                                        start=(dc == 0), stop=(dc == DC - 1),
                                    )
                            for oh in range(OH):
                                os_ = outs_pool.tile([128, OHW], F32, tag="outs", bufs=2)
                                nc.vector.tensor_scalar_mul(out=os_, in0=ps_os[oh],
                                                            scalar1=recips[qc])
                                nc.sync.dma_start(
                                    out=out_d[
                                        q0 + qc * 128 : q0 + (qc + 1) * 128,
                                        oh * OHW : (oh + 1) * OHW,
                                    ],
                                    in_=os_,
                                )

        top.close()

    nc.compile()
    return nc


_NC_CACHE = {}


def _get_nc():
    key = "full"
    if key not in _NC_CACHE:
        _NC_CACHE[key] = build_attn()
    return _NC_CACHE[key]


def prepare_weights(w):
    """Host-side weight prep: M = Wq Wk^T (f32) and Wv, both [D, O]."""
    m = (w[0].astype(np.float64) @ w[1].T.astype(np.float64)).astype(np.float32)
    return np.ascontiguousarray(m), np.ascontiguousarray(w[2], np.float32)


def kernel(**inputs):
    """Full-input entry point: x [8, 2048, 1024], kernel [3, 1024, 1024]."""
    from concourse.bass_utils import run_bass_kernel_spmd

    x = np.ascontiguousarray(inputs["x"], dtype=np.float32)
    w = np.ascontiguousarray(inputs["kernel"], dtype=np.float32)
    B = x.shape[0]
    m, wv = prepare_weights(w)
    nc = _get_nc()
    in_maps = [{"x": x[b], "m": m, "wv": wv} for b in range(B)]
    res = run_bass_kernel_spmd(nc, in_maps, core_ids=list(range(B)))
    return np.stack([res.results[b]["out"] for b in range(B)], axis=0)
